# revision 9
# baseline (speedup 1.0000x reference)
"""DCNv2 deformable RoI pooling on 8 Trainium2 NeuronCores.

Strategy (roi-sharded, host pre-gather + bf16 matmul reduce):
  - Host: replicate the reference's f32 sampling math from (rois, offset),
    derive for each roi a tight rectangular feature-map window and a dense
    separable weight matrix Wmat[px, 49] folding bilinear weights, validity
    and 1/count:  out[n, c, bin] = sum_px Fwin[px, c] * Wmat[px, bin].
  - Host packs, per core (16 rois), every window pixel row as
    [256 bf16 channels | 49 bf16 wmat | 15 pad] = 320 cols (640 B) into ONE
    dense DRAM buffer.  All device DMAs are large contiguous streams.
  - Rois are sorted by window pixel count and dealt round-robin to the 8
    cores so slot s has identical (compile-time) row counts on every core —
    run_bass_kernel_spmd runs one program on all cores; only data differs.
  - Device per core: per slot, one or two big DMAs (HWDGE, alternating
    sync/scalar rings) land the packed rows in SBUF as [px(partitions),
    cols]; per 128-row chunk one matmul with the [K,49] wmat slice as the
    STATIONARY operand streams the 256 bf16 channel cols into psum[49, 256]
    fp32 (accumulated over chunks); DVE copies psum -> bf16 out staging;
    one DMA out.
  - Host: reassemble [128, 256, 7, 7] as float32.
"""
import sys

sys.path.insert(0, "/opt/trn_rl_repo")

import numpy as np
import ml_dtypes

bf16 = ml_dtypes.bfloat16
f32 = np.float32

SPATIAL_SCALE = 0.0625
POOLED = 7
SAMPLE = 4
TRANS_STD = 0.1
B, C, H, W = 2, 256, 160, 160
N_ROIS = 128
NCORES = 8
RPB = N_ROIS // NCORES  # rois per core (= slots)
P, S = POOLED, SAMPLE
NBINS = P * P
PKC = 320  # packed row cols: 256 win + 49 wmat + 15 pad (640B, 64B-aligned)


# ----------------------------------------------------------------- host plan

def _sample_math(rois, offset):
    rois = rois.astype(f32)
    offset = offset.astype(f32)
    b = rois[:, 0].astype(np.int32)
    x1, y1, x2, y2 = rois[:, 1], rois[:, 2], rois[:, 3], rois[:, 4]
    rsw = (np.round(x1) * f32(SPATIAL_SCALE) - f32(0.5)).astype(f32)
    rsh = (np.round(y1) * f32(SPATIAL_SCALE) - f32(0.5)).astype(f32)
    rew = ((np.round(x2) + f32(1.0)) * f32(SPATIAL_SCALE) - f32(0.5)).astype(f32)
    reh = ((np.round(y2) + f32(1.0)) * f32(SPATIAL_SCALE) - f32(0.5)).astype(f32)
    rw = np.maximum(rew - rsw, f32(0.1))
    rh = np.maximum(reh - rsh, f32(0.1))
    bw, bh = (rw / P).astype(f32), (rh / P).astype(f32)
    sw, sh = (bw / S).astype(f32), (bh / S).astype(f32)
    tx = offset[:, 0] * f32(TRANS_STD)
    ty = offset[:, 1] * f32(TRANS_STD)
    pw_i = np.arange(P, dtype=f32)
    ph_i = np.arange(P, dtype=f32)
    wstart = (pw_i[None, None, :] * bw[:, None, None] + rsw[:, None, None]
              + tx * rw[:, None, None]).astype(f32)
    hstart = (ph_i[None, :, None] * bh[:, None, None] + rsh[:, None, None]
              + ty * rh[:, None, None]).astype(f32)
    iw = np.arange(S, dtype=f32)
    x = (wstart[..., None] + iw * sw[:, None, None, None]).astype(f32)
    y = (hstart[..., None] + iw * sh[:, None, None, None]).astype(f32)
    validx = (x >= -0.5) & (x <= W - 0.5)
    validy = (y >= -0.5) & (y <= H - 0.5)
    xc = np.clip(x, f32(0.0), f32(W - 1.0))
    yc = np.clip(y, f32(0.0), f32(H - 1.0))
    x0 = np.floor(xc); x1c = np.ceil(xc)
    y0 = np.floor(yc); y1c = np.ceil(yc)
    dx = (xc - x0).astype(f32)
    dy = (yc - y0).astype(f32)
    cnt = (validx.sum(-1) * validy.sum(-1)).astype(f32)
    denom = np.maximum(cnt, f32(1.0))
    return dict(b=b, validx=validx, validy=validy,
                x0=x0.astype(np.int32), x1=x1c.astype(np.int32),
                y0=y0.astype(np.int32), y1=y1c.astype(np.int32),
                dx=dx, dy=dy, denom=denom)


def _plan(rois, offset):
    sm = _sample_math(rois, offset)
    nroi = sm["b"].shape[0]
    xmin = np.zeros(nroi, np.int64); xmax = np.zeros(nroi, np.int64)
    ymin = np.zeros(nroi, np.int64); ymax = np.zeros(nroi, np.int64)
    vx, vy = sm["validx"], sm["validy"]
    for n in range(nroi):
        joint = (vx[n].any(-1) & vy[n].any(-1))
        if not joint.any():
            continue
        selx = vx[n] & joint[..., None]
        sely = vy[n] & joint[..., None]
        xmin[n] = sm["x0"][n][selx].min(); xmax[n] = sm["x1"][n][selx].max()
        ymin[n] = sm["y0"][n][sely].min(); ymax[n] = sm["y1"][n][sely].max()
    h_need = ymax - ymin + 1
    w_need = xmax - xmin + 1
    px = h_need * w_need

    order = np.argsort(-px, kind="stable")
    # per slot: nch equal chunks of K rows (K*nch >= max px in slot, pad<nch)
    slot_px = []; slot_nch = []; slot_k = []
    for s in range(RPB):
        grp = order[s * NCORES:(s + 1) * NCORES]
        pxs = int(px[grp].max())
        nch = -(-pxs // 128)
        K = -(-pxs // nch)
        slot_px.append(pxs); slot_nch.append(nch); slot_k.append(K)

    # per-roi wmat [px_n, 49] f32 (separable Ay x Bx / denom)
    wmats = {}
    for n in range(nroi):
        h, w = int(h_need[n]), int(w_need[n])
        Ay = np.zeros((NBINS, h), f32)
        Bx = np.zeros((NBINS, w), f32)
        vxn = sm["validx"][n].reshape(NBINS, S)
        vyn = sm["validy"][n].reshape(NBINS, S)
        x0 = sm["x0"][n].reshape(NBINS, S) - xmin[n]
        x1 = sm["x1"][n].reshape(NBINS, S) - xmin[n]
        y0 = sm["y0"][n].reshape(NBINS, S) - ymin[n]
        y1 = sm["y1"][n].reshape(NBINS, S) - ymin[n]
        dx = sm["dx"][n].reshape(NBINS, S)
        dy = sm["dy"][n].reshape(NBINS, S)
        bins = np.repeat(np.arange(NBINS), S)
        np.add.at(Bx, (bins, np.clip(x0, 0, w - 1).ravel()), ((1 - dx) * vxn).ravel())
        np.add.at(Bx, (bins, np.clip(x1, 0, w - 1).ravel()), (dx * vxn).ravel())
        np.add.at(Ay, (bins, np.clip(y0, 0, h - 1).ravel()), ((1 - dy) * vyn).ravel())
        np.add.at(Ay, (bins, np.clip(y1, 0, h - 1).ravel()), (dy * vyn).ravel())
        Wpx = Ay[:, :, None] * Bx[:, None, :] / sm["denom"][n].reshape(NBINS, 1, 1)
        wmats[n] = Wpx.reshape(NBINS, h * w).T.astype(f32)

    return dict(sm=sm, order=order, slot_px=slot_px, slot_nch=slot_nch,
                slot_k=slot_k,
                xmin=xmin, ymin=ymin, h_need=h_need, w_need=w_need,
                wmats=wmats)


# --------------------------------------------------------------- bass program

_PROGRAM_CACHE = {}


N_WARMUP = 24  # PE warmup matmuls (~5us at cold clock) to trip HAM to 2.4GHz


def _build_program(slot_nch, slot_k):
    import concourse.bass as bass
    import concourse.bacc as bacc
    import concourse.mybir as mybir
    import concourse.tile as tile

    # DRAM pack: per slot, [K, nch*PKC] row-major (partition-major layout:
    # each of the K partition rows is one contiguous nch*640B run)
    slot_cols = [n * PKC for n in slot_nch]
    slot_elems = [slot_k[s] * slot_cols[s] for s in range(RPB)]
    tot_elems = sum(slot_elems)
    TW = max(slot_cols)  # per-slot SBUF tile free width

    nc = bacc.Bacc("TRN2", target_bir_lowering=False, debug=False,
                   num_devices=NCORES)
    pack = nc.declare_dram_parameter("pack", [tot_elems],
                                     mybir.dt.bfloat16, isOutput=False)
    out = nc.declare_dram_parameter("out", [NBINS * RPB * C],
                                    mybir.dt.bfloat16, isOutput=True)

    with tile.TileContext(nc) as tc:
        with (
            tc.tile_pool(name="winp", bufs=RPB) as winp,
            tc.tile_pool(name="ostp", bufs=1) as ostp,
            tc.tile_pool(name="wup", bufs=1) as wup,
            tc.tile_pool(name="psum", bufs=7, space="PSUM") as psump,
            tc.tile_pool(name="wpsum", bufs=1, space="PSUM") as wpsump,
        ):
            ostage = ostp.tile([NBINS, RPB * C], mybir.dt.bfloat16)

            # PE warmup: matmuls on an uninitialized scratch tile; results
            # land in a scratch psum tile nothing reads.  Keeps the PE busy
            # from program start so HAM unthrottles before the real matmuls.
            wtile = wup.tile([128, 256], mybir.dt.bfloat16)
            nc.gpsimd.memset(wtile[:], 1.0)
            wpt = wpsump.tile([128, C], mybir.dt.float32, tag="wu")
            for _ in range(N_WARMUP):
                nc.tensor.matmul(wpt[0:NBINS, :], wtile[0:128, 0:NBINS],
                                 wtile[0:128, 0:256], start=True, stop=True)

            rings = [nc.sync, nc.scalar]
            ring_bytes = [0, 0]
            wins = []
            elem0 = 0
            for s in range(RPB):
                K, cols = slot_k[s], slot_cols[s]
                win = winp.tile([128, TW], mybir.dt.bfloat16, tag="win")
                r = 0 if ring_bytes[0] <= ring_bytes[1] else 1
                ring_bytes[r] += slot_elems[s]
                dst = bass.AP(win[:].tensor, win[:].offset,
                              [[TW, K], [1, cols]])
                src = bass.AP(pack[:].tensor, elem0, [[cols, K], [1, cols]])
                rings[r].dma_start(dst, src)
                wins.append(win)
                elem0 += slot_elems[s]

            for s in range(RPB):
                K, nch = slot_k[s], slot_nch[s]
                win = wins[s]
                pt = psump.tile([128, C], mybir.dt.float32, tag="pt")
                for k in range(nch):
                    nc.tensor.matmul(
                        pt[0:NBINS, :],
                        win[0:K, k * PKC + 256:k * PKC + 256 + NBINS],
                        win[0:K, k * PKC:k * PKC + 256],
                        start=(k == 0), stop=(k == nch - 1),
                    )
                nc.vector.tensor_copy(ostage[:, s * C:(s + 1) * C],
                                      pt[0:NBINS, :])
                if s == RPB // 2 - 1:
                    half = (RPB // 2) * C
                    osrc = bass.AP(ostage[:].tensor, ostage[:].offset,
                                   [[RPB * C, NBINS], [1, half]])
                    odst = bass.AP(out[:].tensor, 0,
                                   [[RPB * C, NBINS], [1, half]])
                    nc.sync.dma_start(odst, osrc)

            half = (RPB // 2) * C
            osrc = bass.AP(ostage[:].tensor, ostage[:].offset + half,
                           [[RPB * C, NBINS], [1, half]])
            odst = bass.AP(out[:].tensor, half,
                           [[RPB * C, NBINS], [1, half]])
            nc.scalar.dma_start(odst, osrc)

    nc.compile()
    return nc


# -------------------------------------------------------------------- kernel

TRACE = False
LAST_RESULTS = None


def kernel(input, rois, offset):
    from concourse.bass_utils import run_bass_kernel_spmd

    input = np.ascontiguousarray(np.asarray(input, f32))
    rois = np.asarray(rois, f32)
    offset = np.asarray(offset, f32)

    pl = _plan(rois, offset)
    order = pl["order"]
    slot_nch, slot_k = pl["slot_nch"], pl["slot_k"]
    slot_elems = [slot_k[s] * slot_nch[s] * PKC for s in range(RPB)]
    tot_elems = sum(slot_elems)

    nhwc = np.ascontiguousarray(np.transpose(input, (0, 2, 3, 1)))
    nhwc16 = nhwc.astype(bf16)

    in_maps = []
    for c in range(NCORES):
        packc = np.zeros(tot_elems, bf16)
        elem0 = 0
        for s in range(RPB):
            n = int(order[s * NCORES + c])
            h, w = int(pl["h_need"][n]), int(pl["w_need"][n])
            y0, x0 = int(pl["ymin"][n]), int(pl["xmin"][n])
            bI = int(pl["sm"]["b"][n])
            rows = h * w
            K, nch = slot_k[s], slot_nch[s]
            rowsbuf = np.zeros((nch * K, PKC), bf16)
            rowsbuf[:rows, 0:C] = \
                nhwc16[bI, y0:y0 + h, x0:x0 + w, :].reshape(rows, C)
            rowsbuf[:rows, C:C + NBINS] = pl["wmats"][n].astype(bf16)
            # partition-major: [K, nch*PKC], row p = chunks' p-th rows
            packc[elem0:elem0 + slot_elems[s]] = \
                rowsbuf.reshape(nch, K, PKC).transpose(1, 0, 2).reshape(-1)
            elem0 += slot_elems[s]
        in_maps.append({"pack": packc})

    key = (tuple(slot_nch), tuple(slot_k))
    if key not in _PROGRAM_CACHE:
        _PROGRAM_CACHE[key] = _build_program(list(slot_nch), list(slot_k))
    nc = _PROGRAM_CACHE[key]

    kwargs = {}
    if TRACE:
        kwargs = dict(trace=True, trace_cores=list(range(NCORES)))
    res = run_bass_kernel_spmd(nc, in_maps, list(range(NCORES)), **kwargs)
    global LAST_RESULTS
    LAST_RESULTS = res

    out_full = np.zeros((N_ROIS, C, NBINS), f32)
    for c in range(NCORES):
        o = res.results[c]["out"].astype(f32).reshape(NBINS, RPB, C)
        for s in range(RPB):
            n = int(order[s * NCORES + c])
            out_full[n] = o[:, s, :].T
    return out_full.reshape(N_ROIS, C, P, P)


# revision 18
# speedup vs baseline: 1.3681x; 1.3681x over previous
"""DCNv2 deformable RoI pooling on 8 Trainium2 NeuronCores.

Strategy (roi-sharded, host pre-gather + bf16 matmul reduce):
  - Host: replicate the reference's f32 sampling math from (rois, offset),
    derive for each roi a tight rectangular feature-map window and a dense
    separable weight matrix Wmat[px, 49] folding bilinear weights, validity
    and 1/count:  out[n, c, bin] = sum_px Fwin[px, c] * Wmat[px, bin].
  - Host packs, per core (16 rois), every window pixel row as
    [256 bf16 channels | 49 bf16 wmat | 15 pad] = 320 cols (640 B) into ONE
    dense DRAM buffer.  All device DMAs are large contiguous streams.
  - Rois are sorted by window pixel count and dealt round-robin to the 8
    cores so slot s has identical (compile-time) row counts on every core —
    run_bass_kernel_spmd runs one program on all cores; only data differs.
  - Device per core: per slot, one or two big DMAs (HWDGE, alternating
    sync/scalar rings) land the packed rows in SBUF as [px(partitions),
    cols]; per 128-row chunk one matmul with the [K,49] wmat slice as the
    STATIONARY operand streams the 256 bf16 channel cols into psum[49, 256]
    fp32 (accumulated over chunks); DVE copies psum -> bf16 out staging;
    one DMA out.
  - Host: reassemble [128, 256, 7, 7] as float32.
"""
import sys

sys.path.insert(0, "/opt/trn_rl_repo")

import numpy as np
import ml_dtypes

bf16 = ml_dtypes.bfloat16
f32 = np.float32

SPATIAL_SCALE = 0.0625
POOLED = 7
SAMPLE = 4
TRANS_STD = 0.1
B, C, H, W = 2, 256, 160, 160
N_ROIS = 128
NCORES = 8
RPB = N_ROIS // NCORES  # rois per core (= slots)
P, S = POOLED, SAMPLE
NBINS = P * P
PKC = 305  # packed row cols: 256 win + 49 wmat (610B per pixel row)


# ----------------------------------------------------------------- host plan

def _sample_math(rois, offset):
    rois = rois.astype(f32)
    offset = offset.astype(f32)
    b = rois[:, 0].astype(np.int32)
    x1, y1, x2, y2 = rois[:, 1], rois[:, 2], rois[:, 3], rois[:, 4]
    rsw = (np.round(x1) * f32(SPATIAL_SCALE) - f32(0.5)).astype(f32)
    rsh = (np.round(y1) * f32(SPATIAL_SCALE) - f32(0.5)).astype(f32)
    rew = ((np.round(x2) + f32(1.0)) * f32(SPATIAL_SCALE) - f32(0.5)).astype(f32)
    reh = ((np.round(y2) + f32(1.0)) * f32(SPATIAL_SCALE) - f32(0.5)).astype(f32)
    rw = np.maximum(rew - rsw, f32(0.1))
    rh = np.maximum(reh - rsh, f32(0.1))
    bw, bh = (rw / P).astype(f32), (rh / P).astype(f32)
    sw, sh = (bw / S).astype(f32), (bh / S).astype(f32)
    tx = offset[:, 0] * f32(TRANS_STD)
    ty = offset[:, 1] * f32(TRANS_STD)
    pw_i = np.arange(P, dtype=f32)
    ph_i = np.arange(P, dtype=f32)
    wstart = (pw_i[None, None, :] * bw[:, None, None] + rsw[:, None, None]
              + tx * rw[:, None, None]).astype(f32)
    hstart = (ph_i[None, :, None] * bh[:, None, None] + rsh[:, None, None]
              + ty * rh[:, None, None]).astype(f32)
    iw = np.arange(S, dtype=f32)
    x = (wstart[..., None] + iw * sw[:, None, None, None]).astype(f32)
    y = (hstart[..., None] + iw * sh[:, None, None, None]).astype(f32)
    validx = (x >= -0.5) & (x <= W - 0.5)
    validy = (y >= -0.5) & (y <= H - 0.5)
    xc = np.clip(x, f32(0.0), f32(W - 1.0))
    yc = np.clip(y, f32(0.0), f32(H - 1.0))
    x0 = np.floor(xc); x1c = np.ceil(xc)
    y0 = np.floor(yc); y1c = np.ceil(yc)
    dx = (xc - x0).astype(f32)
    dy = (yc - y0).astype(f32)
    cnt = (validx.sum(-1) * validy.sum(-1)).astype(f32)
    denom = np.maximum(cnt, f32(1.0))
    return dict(b=b, validx=validx, validy=validy,
                x0=x0.astype(np.int32), x1=x1c.astype(np.int32),
                y0=y0.astype(np.int32), y1=y1c.astype(np.int32),
                dx=dx, dy=dy, denom=denom)


def _plan(rois, offset):
    sm = _sample_math(rois, offset)
    nroi = sm["b"].shape[0]
    xmin = np.zeros(nroi, np.int64); xmax = np.zeros(nroi, np.int64)
    ymin = np.zeros(nroi, np.int64); ymax = np.zeros(nroi, np.int64)
    vx, vy = sm["validx"], sm["validy"]
    for n in range(nroi):
        joint = (vx[n].any(-1) & vy[n].any(-1))
        if not joint.any():
            continue
        selx = vx[n] & joint[..., None]
        sely = vy[n] & joint[..., None]
        xmin[n] = sm["x0"][n][selx].min(); xmax[n] = sm["x1"][n][selx].max()
        ymin[n] = sm["y0"][n][sely].min(); ymax[n] = sm["y1"][n][sely].max()
    h_need = ymax - ymin + 1
    w_need = xmax - xmin + 1
    px = h_need * w_need

    order = np.argsort(px, kind="stable")  # ascending: small slots first
    # per slot: nch chunks of 128 rows (full partition coverage keeps the
    # per-partition descriptor->SDMA-engine load even across all 16 engines)
    slot_px = []; slot_nch = []
    for s in range(RPB):
        grp = order[s * NCORES:(s + 1) * NCORES]
        pxs = int(px[grp].max())
        slot_px.append(pxs); slot_nch.append(-(-pxs // 128))

    # per-roi wmat [px_n, 49] f32 (separable Ay x Bx / denom)
    wmats = {}
    for n in range(nroi):
        h, w = int(h_need[n]), int(w_need[n])
        Ay = np.zeros((NBINS, h), f32)
        Bx = np.zeros((NBINS, w), f32)
        vxn = sm["validx"][n].reshape(NBINS, S)
        vyn = sm["validy"][n].reshape(NBINS, S)
        x0 = sm["x0"][n].reshape(NBINS, S) - xmin[n]
        x1 = sm["x1"][n].reshape(NBINS, S) - xmin[n]
        y0 = sm["y0"][n].reshape(NBINS, S) - ymin[n]
        y1 = sm["y1"][n].reshape(NBINS, S) - ymin[n]
        dx = sm["dx"][n].reshape(NBINS, S)
        dy = sm["dy"][n].reshape(NBINS, S)
        bins = np.repeat(np.arange(NBINS), S)
        np.add.at(Bx, (bins, np.clip(x0, 0, w - 1).ravel()), ((1 - dx) * vxn).ravel())
        np.add.at(Bx, (bins, np.clip(x1, 0, w - 1).ravel()), (dx * vxn).ravel())
        np.add.at(Ay, (bins, np.clip(y0, 0, h - 1).ravel()), ((1 - dy) * vyn).ravel())
        np.add.at(Ay, (bins, np.clip(y1, 0, h - 1).ravel()), (dy * vyn).ravel())
        Wpx = Ay[:, :, None] * Bx[:, None, :] / sm["denom"][n].reshape(NBINS, 1, 1)
        wmats[n] = Wpx.reshape(NBINS, h * w).T.astype(f32)

    return dict(sm=sm, order=order, slot_px=slot_px, slot_nch=slot_nch,
                xmin=xmin, ymin=ymin, h_need=h_need, w_need=w_need,
                wmats=wmats)


# --------------------------------------------------------------- bass program

_PROGRAM_CACHE = {}


N_WARMUP = 20  # PE warmup matmuls (~4.3us at cold clock) to trip HAM to 2.4GHz
NPAIR = RPB // 2


def _build_program(slot_nch):
    import concourse.bass as bass
    import concourse.bacc as bacc
    import concourse.mybir as mybir
    import concourse.tile as tile

    # DRAM pack: per slot, [128, nch*PKC] row-major (partition-major layout:
    # each of the 128 partition rows is one contiguous nch*610B run)
    slot_cols = [n * PKC for n in slot_nch]
    slot_elems = [128 * c for c in slot_cols]
    tot_elems = sum(slot_elems)
    TW = max(slot_cols)  # per-slot SBUF tile free width

    nc = bacc.Bacc("TRN2", target_bir_lowering=False, debug=False,
                   num_devices=NCORES)
    pack = nc.declare_dram_parameter("pack", [tot_elems],
                                     mybir.dt.bfloat16, isOutput=False)
    out = nc.declare_dram_parameter("out", [128 * NPAIR * C],
                                    mybir.dt.bfloat16, isOutput=True)

    with tile.TileContext(nc) as tc:
        with (
            tc.tile_pool(name="winp", bufs=RPB) as winp,
            tc.tile_pool(name="ostp", bufs=1) as ostp,
            tc.tile_pool(name="wup", bufs=1) as wup,
            tc.tile_pool(name="psum", bufs=8, space="PSUM") as psump,
        ):
            # pair p of slots (2p, 2p+1): even slot -> psum/ostage partitions
            # 0..48, odd slot -> 64..112 (distinct PE column groups; also
            # spreads the out DMA across all 16 SDMA engines)
            ostage = ostp.tile([128, NPAIR * C], mybir.dt.bfloat16)
            nc.gpsimd.memset(ostage[:], 0.0)

            # PE warmup: matmuls on a memset scratch tile; results land in a
            # recycled psum tile nothing reads.  Keeps the PE busy from
            # program start so HAM unthrottles before the real matmuls.
            wtile = wup.tile([128, 256], mybir.dt.bfloat16)
            nc.gpsimd.memset(wtile[:], 1.0)
            wpt = psump.tile([128, C], mybir.dt.float32, tag="pt")
            for _ in range(N_WARMUP):
                nc.tensor.matmul(wpt[0:NBINS, :], wtile[0:128, 0:NBINS],
                                 wtile[0:128, 0:256], start=True, stop=True)

            rings = [nc.sync, nc.scalar]
            ring_bytes = [0, 0]
            wins = []
            elem0 = 0
            for s in range(RPB):
                cols = slot_cols[s]
                win = winp.tile([128, TW], mybir.dt.bfloat16, tag="win")
                r = 0 if ring_bytes[0] <= ring_bytes[1] else 1
                ring_bytes[r] += slot_elems[s]
                dst = bass.AP(win[:].tensor, win[:].offset,
                              [[TW, 128], [1, cols]])
                src = bass.AP(pack[:].tensor, elem0, [[cols, 128], [1, cols]])
                rings[r].dma_start(dst, src)
                wins.append(win)
                elem0 += slot_elems[s]

            for p in range(NPAIR):
                sA, sB = 2 * p, 2 * p + 1
                nchA, nchB = slot_nch[sA], slot_nch[sB]
                winA, winB = wins[sA], wins[sB]
                pt = psump.tile([128, C], mybir.dt.float32, tag="pt")
                for k in range(max(nchA, nchB)):
                    if k < nchA:
                        nc.tensor.matmul(
                            pt[0:NBINS, :],
                            winA[:, k * PKC + 256:k * PKC + 256 + NBINS],
                            winA[:, k * PKC:k * PKC + 256],
                            start=(k == 0), stop=(k == nchA - 1),
                            tile_position=(0, 0),
                        )
                    if k < nchB:
                        nc.tensor.matmul(
                            pt[64:64 + NBINS, :],
                            winB[:, k * PKC + 256:k * PKC + 256 + NBINS],
                            winB[:, k * PKC:k * PKC + 256],
                            start=(k == 0), stop=(k == nchB - 1),
                            tile_position=(0, 64),
                        )
                nc.vector.tensor_copy(
                    ostage[0:NBINS, p * C:(p + 1) * C], pt[0:NBINS, :])
                nc.scalar.copy(
                    ostage[64:64 + NBINS, p * C:(p + 1) * C],
                    pt[64:64 + NBINS, :])
                if p == NPAIR // 2 - 1:
                    half = (NPAIR // 2) * C
                    osrc = bass.AP(ostage[:].tensor, ostage[:].offset,
                                   [[NPAIR * C, 128], [1, half]])
                    odst = bass.AP(out[:].tensor, 0,
                                   [[NPAIR * C, 128], [1, half]])
                    nc.sync.dma_start(odst, osrc)

            half = (NPAIR // 2) * C
            osrc = bass.AP(ostage[:].tensor, ostage[:].offset + half,
                           [[NPAIR * C, 128], [1, half]])
            odst = bass.AP(out[:].tensor, half,
                           [[NPAIR * C, 128], [1, half]])
            nc.scalar.dma_start(odst, osrc)

    nc.compile()
    return nc


# -------------------------------------------------------------------- kernel

TRACE = False
LAST_RESULTS = None


def kernel(input, rois, offset):
    from concourse.bass_utils import run_bass_kernel_spmd

    input = np.ascontiguousarray(np.asarray(input, f32))
    rois = np.asarray(rois, f32)
    offset = np.asarray(offset, f32)

    pl = _plan(rois, offset)
    order = pl["order"]
    slot_nch = pl["slot_nch"]
    slot_elems = [128 * slot_nch[s] * PKC for s in range(RPB)]
    tot_elems = sum(slot_elems)

    nhwc = np.ascontiguousarray(np.transpose(input, (0, 2, 3, 1)))
    nhwc16 = nhwc.astype(bf16)

    in_maps = []
    for c in range(NCORES):
        packc = np.zeros(tot_elems, bf16)
        elem0 = 0
        for s in range(RPB):
            n = int(order[s * NCORES + c])
            h, w = int(pl["h_need"][n]), int(pl["w_need"][n])
            y0, x0 = int(pl["ymin"][n]), int(pl["xmin"][n])
            bI = int(pl["sm"]["b"][n])
            rows = h * w
            nch = slot_nch[s]
            rowsbuf = np.zeros((nch * 128, PKC), bf16)
            rowsbuf[:rows, 0:C] = \
                nhwc16[bI, y0:y0 + h, x0:x0 + w, :].reshape(rows, C)
            rowsbuf[:rows, C:C + NBINS] = pl["wmats"][n].astype(bf16)
            # partition-major: [128, nch*PKC], row p = chunks' p-th rows
            packc[elem0:elem0 + slot_elems[s]] = \
                rowsbuf.reshape(nch, 128, PKC).transpose(1, 0, 2).reshape(-1)
            elem0 += slot_elems[s]
        in_maps.append({"pack": packc})

    key = tuple(slot_nch)
    if key not in _PROGRAM_CACHE:
        _PROGRAM_CACHE[key] = _build_program(list(slot_nch))
    nc = _PROGRAM_CACHE[key]

    kwargs = {}
    if TRACE:
        kwargs = dict(trace=True, trace_cores=list(range(NCORES)))
    res = run_bass_kernel_spmd(nc, in_maps, list(range(NCORES)), **kwargs)
    global LAST_RESULTS
    LAST_RESULTS = res

    out_full = np.zeros((N_ROIS, C, NBINS), f32)
    for c in range(NCORES):
        o = res.results[c]["out"].astype(f32).reshape(128, RPB // 2, C)
        for s in range(RPB):
            n = int(order[s * NCORES + c])
            p, r = s // 2, s % 2
            out_full[n] = o[64 * r:64 * r + NBINS, p, :].T
    return out_full.reshape(N_ROIS, C, P, P)


# revision 21
# speedup vs baseline: 1.6002x; 1.1697x over previous
"""DCNv2 deformable RoI pooling on 8 Trainium2 NeuronCores.

Strategy (roi-sharded, host pre-gather + bf16 matmul reduce):
  - Host: replicate the reference's f32 sampling math from (rois, offset),
    derive for each roi a tight rectangular feature-map window and a dense
    separable weight matrix Wmat[px, 49] folding bilinear weights, validity
    and 1/count:  out[n, c, bin] = sum_px Fwin[px, c] * Wmat[px, bin].
  - Host packs, per core (16 rois), every window pixel row as
    [256 bf16 channels | 49 bf16 wmat | 15 pad] = 320 cols (640 B) into ONE
    dense DRAM buffer.  All device DMAs are large contiguous streams.
  - Rois are sorted by window pixel count and dealt round-robin to the 8
    cores so slot s has identical (compile-time) row counts on every core —
    run_bass_kernel_spmd runs one program on all cores; only data differs.
  - Device per core: per slot, one or two big DMAs (HWDGE, alternating
    sync/scalar rings) land the packed rows in SBUF as [px(partitions),
    cols]; per 128-row chunk one matmul with the [K,49] wmat slice as the
    STATIONARY operand streams the 256 bf16 channel cols into psum[49, 256]
    fp32 (accumulated over chunks); DVE copies psum -> bf16 out staging;
    one DMA out.
  - Host: reassemble [128, 256, 7, 7] as float32.
"""
import sys

sys.path.insert(0, "/opt/trn_rl_repo")

import numpy as np
import ml_dtypes

bf16 = ml_dtypes.bfloat16
f32 = np.float32

SPATIAL_SCALE = 0.0625
POOLED = 7
SAMPLE = 4
TRANS_STD = 0.1
B, C, H, W = 2, 256, 160, 160
N_ROIS = 128
NCORES = 8
RPB = N_ROIS // NCORES  # rois per core (= slots)
P, S = POOLED, SAMPLE
NBINS = P * P
PKC = 305  # packed row cols: 256 win + 49 wmat (610B per pixel row)


# ----------------------------------------------------------------- host plan

def _sample_math(rois, offset):
    rois = rois.astype(f32)
    offset = offset.astype(f32)
    b = rois[:, 0].astype(np.int32)
    x1, y1, x2, y2 = rois[:, 1], rois[:, 2], rois[:, 3], rois[:, 4]
    rsw = (np.round(x1) * f32(SPATIAL_SCALE) - f32(0.5)).astype(f32)
    rsh = (np.round(y1) * f32(SPATIAL_SCALE) - f32(0.5)).astype(f32)
    rew = ((np.round(x2) + f32(1.0)) * f32(SPATIAL_SCALE) - f32(0.5)).astype(f32)
    reh = ((np.round(y2) + f32(1.0)) * f32(SPATIAL_SCALE) - f32(0.5)).astype(f32)
    rw = np.maximum(rew - rsw, f32(0.1))
    rh = np.maximum(reh - rsh, f32(0.1))
    bw, bh = (rw / P).astype(f32), (rh / P).astype(f32)
    sw, sh = (bw / S).astype(f32), (bh / S).astype(f32)
    tx = offset[:, 0] * f32(TRANS_STD)
    ty = offset[:, 1] * f32(TRANS_STD)
    pw_i = np.arange(P, dtype=f32)
    ph_i = np.arange(P, dtype=f32)
    wstart = (pw_i[None, None, :] * bw[:, None, None] + rsw[:, None, None]
              + tx * rw[:, None, None]).astype(f32)
    hstart = (ph_i[None, :, None] * bh[:, None, None] + rsh[:, None, None]
              + ty * rh[:, None, None]).astype(f32)
    iw = np.arange(S, dtype=f32)
    x = (wstart[..., None] + iw * sw[:, None, None, None]).astype(f32)
    y = (hstart[..., None] + iw * sh[:, None, None, None]).astype(f32)
    validx = (x >= -0.5) & (x <= W - 0.5)
    validy = (y >= -0.5) & (y <= H - 0.5)
    xc = np.clip(x, f32(0.0), f32(W - 1.0))
    yc = np.clip(y, f32(0.0), f32(H - 1.0))
    x0 = np.floor(xc); x1c = np.ceil(xc)
    y0 = np.floor(yc); y1c = np.ceil(yc)
    dx = (xc - x0).astype(f32)
    dy = (yc - y0).astype(f32)
    cnt = (validx.sum(-1) * validy.sum(-1)).astype(f32)
    denom = np.maximum(cnt, f32(1.0))
    return dict(b=b, validx=validx, validy=validy,
                x0=x0.astype(np.int32), x1=x1c.astype(np.int32),
                y0=y0.astype(np.int32), y1=y1c.astype(np.int32),
                dx=dx, dy=dy, denom=denom)


def _plan(rois, offset):
    sm = _sample_math(rois, offset)
    nroi = sm["b"].shape[0]
    xmin = np.zeros(nroi, np.int64); xmax = np.zeros(nroi, np.int64)
    ymin = np.zeros(nroi, np.int64); ymax = np.zeros(nroi, np.int64)
    vx, vy = sm["validx"], sm["validy"]
    for n in range(nroi):
        joint = (vx[n].any(-1) & vy[n].any(-1))
        if not joint.any():
            continue
        selx = vx[n] & joint[..., None]
        sely = vy[n] & joint[..., None]
        xmin[n] = sm["x0"][n][selx].min(); xmax[n] = sm["x1"][n][selx].max()
        ymin[n] = sm["y0"][n][sely].min(); ymax[n] = sm["y1"][n][sely].max()
    h_need = ymax - ymin + 1
    w_need = xmax - xmin + 1
    px = h_need * w_need

    order = np.argsort(-px, kind="stable")  # descending: big slots first
    # per slot: nch chunks of 128 rows (full partition coverage keeps the
    # per-partition descriptor->SDMA-engine load even across all 16 engines)
    slot_px = []; slot_nch = []
    for s in range(RPB):
        grp = order[s * NCORES:(s + 1) * NCORES]
        pxs = int(px[grp].max())
        slot_px.append(pxs); slot_nch.append(-(-pxs // 128))

    # per-roi wmat [px_n, 49] f32 (separable Ay x Bx / denom)
    wmats = {}
    for n in range(nroi):
        h, w = int(h_need[n]), int(w_need[n])
        Ay = np.zeros((NBINS, h), f32)
        Bx = np.zeros((NBINS, w), f32)
        vxn = sm["validx"][n].reshape(NBINS, S)
        vyn = sm["validy"][n].reshape(NBINS, S)
        x0 = sm["x0"][n].reshape(NBINS, S) - xmin[n]
        x1 = sm["x1"][n].reshape(NBINS, S) - xmin[n]
        y0 = sm["y0"][n].reshape(NBINS, S) - ymin[n]
        y1 = sm["y1"][n].reshape(NBINS, S) - ymin[n]
        dx = sm["dx"][n].reshape(NBINS, S)
        dy = sm["dy"][n].reshape(NBINS, S)
        bins = np.repeat(np.arange(NBINS), S)
        np.add.at(Bx, (bins, np.clip(x0, 0, w - 1).ravel()), ((1 - dx) * vxn).ravel())
        np.add.at(Bx, (bins, np.clip(x1, 0, w - 1).ravel()), (dx * vxn).ravel())
        np.add.at(Ay, (bins, np.clip(y0, 0, h - 1).ravel()), ((1 - dy) * vyn).ravel())
        np.add.at(Ay, (bins, np.clip(y1, 0, h - 1).ravel()), (dy * vyn).ravel())
        Wpx = Ay[:, :, None] * Bx[:, None, :] / sm["denom"][n].reshape(NBINS, 1, 1)
        wmats[n] = Wpx.reshape(NBINS, h * w).T.astype(f32)

    return dict(sm=sm, order=order, slot_px=slot_px, slot_nch=slot_nch,
                xmin=xmin, ymin=ymin, h_need=h_need, w_need=w_need,
                wmats=wmats)


# --------------------------------------------------------------- bass program

_PROGRAM_CACHE = {}


N_WARMUP = 20  # PE warmup matmuls (~4.3us at cold clock) to trip HAM to 2.4GHz
NPAIR = RPB // 2


def _build_program(slot_nch):
    import concourse.bass as bass
    import concourse.bacc as bacc
    import concourse.mybir as mybir
    import concourse.tile as tile

    # DRAM pack: per slot, [128, nch*PKC] row-major (partition-major layout:
    # each of the 128 partition rows is one contiguous nch*610B run)
    slot_cols = [n * PKC for n in slot_nch]
    slot_elems = [128 * c for c in slot_cols]
    tot_elems = sum(slot_elems)
    TW = max(slot_cols)  # per-slot SBUF tile free width

    nc = bacc.Bacc("TRN2", target_bir_lowering=False, debug=False,
                   num_devices=NCORES)
    pack = nc.declare_dram_parameter("pack", [tot_elems],
                                     mybir.dt.bfloat16, isOutput=False)
    out = nc.declare_dram_parameter("out", [128 * NPAIR * C],
                                    mybir.dt.bfloat16, isOutput=True)

    # group consecutive same-nch slots: one DMA instruction per group
    groups = []  # (slot0, n_g, nch)
    s = 0
    while s < RPB:
        e = s
        while e < RPB and slot_nch[e] == slot_nch[s]:
            e += 1
        groups.append((s, e - s, slot_nch[s]))
        s = e

    with tile.TileContext(nc) as tc:
        with (
            tc.tile_pool(name="winp", bufs=1) as winp,
            tc.tile_pool(name="ostp", bufs=1) as ostp,
            tc.tile_pool(name="wup", bufs=1) as wup,
            tc.tile_pool(name="psum", bufs=8, space="PSUM") as psump,
        ):
            # pair p of slots (2p, 2p+1): even slot -> psum/ostage partitions
            # 0..48, odd slot -> 64..112 (distinct PE column groups; also
            # spreads the out DMA across all 16 SDMA engines)
            ostage = ostp.tile([128, NPAIR * C], mybir.dt.bfloat16)
            nc.gpsimd.memset(ostage[:], 0.0)

            # PE warmup: matmuls on a memset scratch tile; results land in a
            # recycled psum tile nothing reads.  Keeps the PE busy from
            # program start so HAM unthrottles before the real matmuls.
            wtile = wup.tile([128, 256], mybir.dt.bfloat16)
            nc.gpsimd.memset(wtile[:], 1.0)
            wpt = psump.tile([128, C], mybir.dt.float32, tag="pt")
            for _ in range(N_WARMUP):
                nc.tensor.matmul(wpt[0:NBINS, :], wtile[0:128, 0:NBINS],
                                 wtile[0:128, 0:256], start=True, stop=True)

            rings = [nc.sync, nc.scalar]
            gtiles = []
            elem0 = 0
            for g, (s0, ng, nch) in enumerate(groups):
                cols = nch * PKC
                GW = ng * cols
                gt = winp.tile([128, GW], mybir.dt.bfloat16, tag=f"g{g}")
                dst = bass.AP(gt[:].tensor, gt[:].offset,
                              [[GW, 128], [cols, ng], [1, cols]])
                src = bass.AP(pack[:].tensor, elem0,
                              [[cols, 128], [128 * cols, ng], [1, cols]])
                rings[g % 2].dma_start(dst, src)
                gtiles.append(gt)
                elem0 += 128 * GW

            def slot_view(s):
                for g, (s0, ng, nch) in enumerate(groups):
                    if s0 <= s < s0 + ng:
                        return gtiles[g], (s - s0) * nch * PKC
                raise AssertionError

            for p in range(NPAIR):
                sA, sB = 2 * p, 2 * p + 1
                nchA, nchB = slot_nch[sA], slot_nch[sB]
                winA, offA = slot_view(sA)
                winB, offB = slot_view(sB)
                pt = psump.tile([128, C], mybir.dt.float32, tag="pt")
                for k in range(max(nchA, nchB)):
                    if k < nchA:
                        c0 = offA + k * PKC
                        nc.tensor.matmul(
                            pt[0:NBINS, :],
                            winA[:, c0 + 256:c0 + 256 + NBINS],
                            winA[:, c0:c0 + 256],
                            start=(k == 0), stop=(k == nchA - 1),
                            tile_position=(0, 0),
                        )
                    if k < nchB:
                        c0 = offB + k * PKC
                        nc.tensor.matmul(
                            pt[64:64 + NBINS, :],
                            winB[:, c0 + 256:c0 + 256 + NBINS],
                            winB[:, c0:c0 + 256],
                            start=(k == 0), stop=(k == nchB - 1),
                            tile_position=(0, 64),
                        )
                nc.vector.tensor_copy(
                    ostage[0:NBINS, p * C:(p + 1) * C], pt[0:NBINS, :])
                nc.scalar.copy(
                    ostage[64:64 + NBINS, p * C:(p + 1) * C],
                    pt[64:64 + NBINS, :])
                if p % 2 == 1:  # out quarter after pairs (0,1), (2,3), ...
                    q = p // 2
                    qc = 2 * C
                    osrc = bass.AP(ostage[:].tensor,
                                   ostage[:].offset + q * qc,
                                   [[NPAIR * C, 128], [1, qc]])
                    odst = bass.AP(out[:].tensor, q * qc,
                                   [[NPAIR * C, 128], [1, qc]])
                    rings[q % 2].dma_start(odst, osrc)

    nc.compile()
    return nc


# -------------------------------------------------------------------- kernel

TRACE = False
LAST_RESULTS = None


def kernel(input, rois, offset):
    from concourse.bass_utils import run_bass_kernel_spmd

    input = np.ascontiguousarray(np.asarray(input, f32))
    rois = np.asarray(rois, f32)
    offset = np.asarray(offset, f32)

    pl = _plan(rois, offset)
    order = pl["order"]
    slot_nch = pl["slot_nch"]
    slot_elems = [128 * slot_nch[s] * PKC for s in range(RPB)]
    tot_elems = sum(slot_elems)

    nhwc = np.ascontiguousarray(np.transpose(input, (0, 2, 3, 1)))
    nhwc16 = nhwc.astype(bf16)

    in_maps = []
    for c in range(NCORES):
        packc = np.zeros(tot_elems, bf16)
        elem0 = 0
        for s in range(RPB):
            n = int(order[s * NCORES + c])
            h, w = int(pl["h_need"][n]), int(pl["w_need"][n])
            y0, x0 = int(pl["ymin"][n]), int(pl["xmin"][n])
            bI = int(pl["sm"]["b"][n])
            rows = h * w
            nch = slot_nch[s]
            rowsbuf = np.zeros((nch * 128, PKC), bf16)
            rowsbuf[:rows, 0:C] = \
                nhwc16[bI, y0:y0 + h, x0:x0 + w, :].reshape(rows, C)
            rowsbuf[:rows, C:C + NBINS] = pl["wmats"][n].astype(bf16)
            # partition-major: [128, nch*PKC], row p = chunks' p-th rows
            packc[elem0:elem0 + slot_elems[s]] = \
                rowsbuf.reshape(nch, 128, PKC).transpose(1, 0, 2).reshape(-1)
            elem0 += slot_elems[s]
        in_maps.append({"pack": packc})

    key = tuple(slot_nch)
    if key not in _PROGRAM_CACHE:
        _PROGRAM_CACHE[key] = _build_program(list(slot_nch))
    nc = _PROGRAM_CACHE[key]

    kwargs = {}
    if TRACE:
        kwargs = dict(trace=True, trace_cores=list(range(NCORES)))
    res = run_bass_kernel_spmd(nc, in_maps, list(range(NCORES)), **kwargs)
    global LAST_RESULTS
    LAST_RESULTS = res

    out_full = np.zeros((N_ROIS, C, NBINS), f32)
    for c in range(NCORES):
        o = res.results[c]["out"].astype(f32).reshape(128, RPB // 2, C)
        for s in range(RPB):
            n = int(order[s * NCORES + c])
            p, r = s // 2, s % 2
            out_full[n] = o[64 * r:64 * r + NBINS, p, :].T
    return out_full.reshape(N_ROIS, C, P, P)
